# revision 3
# baseline (speedup 1.0000x reference)
"""BERT attention (QKV proj + SDPA) sharded over 8 trn2 NeuronCores by head.

Problem: hidden_states [2, 2048, 1024], 16 heads x 64 dim, fp32.
Sharding: 2 heads per core (tensor-parallel on Q/K/V weight columns).

Math shortcuts (exact):
  - bk drops out entirely (softmax is invariant to per-row score shifts).
  - bv is added on the HOST after the gather (ctx = softmax(P)@(V+bv) =
    softmax(P)@V + bv).

Per-core device kernel:
  inputs:  xh/xr [1024, 4096] fp8e4m3  X^T two-term split (x = xh + xr)
           wh*/wr* [1024, 128] fp8     weight two-term split at common
                                       scale 512: W ~ (wh*16 + wr)/512
           bq [128, 1] f32             q bias slice
  output:  out [4096, 128] f32         context for this core's 2 heads

Dataflow per batch:
  1. Q/K projections as 3-pass fp8 DoubleRow matmuls (contraction 256/instr,
     0.5 cyc/col): psum = xh@wh16 + xr@wh16 + xh@wr (all scale-512), then
     DVE copy psum*(1/512)(+bq) -> fp16 qt/kt  (k needs no bias).
  2. V computed DIRECTLY in [token, dim] layout (stationary = X^T tile,
     moving = wv): no PE transposes. DVE copies psum*(1/512) into the
     ones-augmented V' tile [128k, kt, head, 65].
  3. Scores TRANSPOSED fp16: ST[k, q] f32 psum, 2 heads x 2 j-halves.
  4. exp: ACT native Exp (scale 1/8) for 3 of 4 (head, j-half) row groups
     per unit; the 4th goes to DVE via the Schraudolph u16-bits trick
     (bits = round(s*1024/(8 ln2) + 15301.5) bitcast fp16). Flavor is
     UNIFORM along k for each q-row so the softmax denominator (ones
     column) cancels each flavor's bias.
  5. P@V with q stationary: ctx[q=128, d|sum 65] = sum_kt PT[k,qc].T@V'[k,65];
     DVE reciprocal of the sums column + tensor_scalar multiply, DMA out.

Fillers (projections, V tiles, deferred P@V) are emitted inside the kt loop
paced by a credit system. PSUM: scores 2x2 banks, proj 2x1, ctx/v 2x1.
"""

import numpy as np
import ml_dtypes

B, S, HID = 2, 2048, 1024
T = B * S
N_CORES = 8
P = 128
D = 64
HC = 4  # hidden chunks of 256 (DoubleRow pairs of 128)

F16 = np.float16
F8 = ml_dtypes.float8_e4m3

_CACHED = {}

# DVE-exp (head, j-half) row-group per unit; remaining 3/4 row groups use
# ACT native exp. Uniform flavor along k per q-row.
DVE_JH = {0: {(0, 0)}, 1: {(1, 1)}, 2: {(0, 1)}, 3: {(1, 0)}}
A16 = 0.125 * 1024.0 / np.log(2.0)
B16 = 15.0 * 1024.0 - 58.5  # centered Schraudolph bias


def _build():
    from collections import deque

    import concourse.bass as bass
    from concourse import bacc
    import concourse.tile as tile
    import concourse.mybir as mybir
    from concourse.bass import ts, ds

    f8 = mybir.dt.float8e4
    f16 = mybir.dt.float16
    f32 = mybir.dt.float32
    u16 = mybir.dt.uint16
    Exp = mybir.ActivationFunctionType.Exp
    DR = mybir.MatmulPerfMode.DoubleRow
    MULT = mybir.AluOpType.mult
    ADD = mybir.AluOpType.add

    nc = bacc.Bacc(trn_type="TRN2", target_bir_lowering=False, debug=False)

    xh = nc.dram_tensor("xh", [HID, T], f8, kind="ExternalInput").ap()
    xr = nc.dram_tensor("xr", [HID, T], f8, kind="ExternalInput").ap()
    wts = {}
    for nm in ("whq", "wrq", "whk", "wrk", "whv", "wrv"):
        wts[nm] = nc.dram_tensor(nm, [HID, P], f8, kind="ExternalInput").ap()
    bq = nc.dram_tensor("bq", [P, 1], f32, kind="ExternalInput").ap()
    out = nc.dram_tensor("out", [T, P], f32, kind="ExternalOutput").ap()

    with tile.TileContext(nc) as tc:
        with (
            tc.tile_pool(name="const", bufs=1) as cpool,
            tc.tile_pool(name="xtp", bufs=1) as xtpool,
            tc.tile_pool(name="qkv", bufs=1) as qkvpool,
            tc.tile_pool(name="pt", bufs=1) as ptpool,
            tc.tile_pool(name="small", bufs=4) as smallpool,
            tc.tile_pool(name="ot", bufs=3) as otpool,
            tc.tile_pool(name="ps", bufs=2, space="PSUM") as psp,
        ):
            # X^T half-buffer (one batch), two fp8 terms; hid = a*256+i*128+p
            xh_sb = xtpool.tile([P, HC, 2, S], f8, tag="xh")
            xr_sb = xtpool.tile([P, HC, 2, S], f8, tag="xr")
            xhp = xh.rearrange("(a i p) t -> p a i t", p=P, i=2)
            xrp = xr.rearrange("(a i p) t -> p a i t", p=P, i=2)
            w_sbs = {}
            bq_sb = cpool.tile([P, 1], f32, tag="bq")
            for nm in wts:
                w_sbs[nm] = cpool.tile([P, HC, 2, P], f8, tag=nm, name=nm)
            # DMA arrival order matched to first-consumption order
            nc.sync.dma_start(xh_sb[:, :, :, 0:512], xhp[:, :, :, 0:512])
            nc.sync.dma_start(bq_sb, bq)
            nc.sync.dma_start(w_sbs["whq"], wts["whq"].rearrange("(a i p) c -> p a i c", p=P, i=2))
            nc.sync.dma_start(w_sbs["wrq"], wts["wrq"].rearrange("(a i p) c -> p a i c", p=P, i=2))
            nc.sync.dma_start(xr_sb[:, :, :, 0:512], xrp[:, :, :, 0:512])
            nc.sync.dma_start(xh_sb[:, :, :, ts(1, 512)], xhp[:, :, :, ts(1, 512)])
            nc.sync.dma_start(xr_sb[:, :, :, ts(1, 512)], xrp[:, :, :, ts(1, 512)])
            nc.sync.dma_start(w_sbs["whk"], wts["whk"].rearrange("(a i p) c -> p a i c", p=P, i=2))
            nc.sync.dma_start(w_sbs["wrk"], wts["wrk"].rearrange("(a i p) c -> p a i c", p=P, i=2))
            nc.sync.dma_start(w_sbs["whv"], wts["whv"].rearrange("(a i p) c -> p a i c", p=P, i=2))
            nc.sync.dma_start(w_sbs["wrv"], wts["wrv"].rearrange("(a i p) c -> p a i c", p=P, i=2))
            for quarter in range(2, 4):
                nc.sync.dma_start(xh_sb[:, :, :, ts(quarter, 512)], xhp[:, :, :, ts(quarter, 512)])
                nc.sync.dma_start(xr_sb[:, :, :, ts(quarter, 512)], xrp[:, :, :, ts(quarter, 512)])

            # zero operand for PE warm-up (values unused)
            garb = cpool.tile([P, 512], f16, tag="garb")
            nc.gpsimd.memset(garb, 0.0)

            qt_sb = qkvpool.tile([P, T], f16, tag="qt")
            kt_sb = qkvpool.tile([P, T], f16, tag="kt")
            # V' [k-part, ktile(32), head(2), 66]; col 64 = ones (row sums)
            vp_sb = qkvpool.tile([P, T // P, 2, D + 2], f16, tag="vp")
            nc.gpsimd.memset(vp_sb[:, :, :, D : D + 1], 1.0)

            # PE warm-up while the first DMAs land: long enough that the
            # p-state ramp completes AND the PE never idles before the first
            # projection.
            wu = psp.tile([P, 512], f32, tag="pj", bufs=2, name="wups")
            NWU = 17
            for i in range(NWU):
                nc.tensor.matmul(
                    wu, garb[:, 0:P], garb, start=(i == 0), stop=(i == NWU - 1)
                )
            nc.vector.tensor_copy(kt_sb[:, 0:512], wu)  # dummy drain (overwritten)

            def proj_group(t8, which):
                """Project 512 tokens (chunk t8) for q or k: 3-pass fp8 DR."""
                wh, wr = w_sbs["wh" + which], w_sbs["wr" + which]
                dst = qt_sb if which == "q" else kt_sb
                ps = psp.tile([P, 512], f32, tag="pj", bufs=2, name="projps")
                for half in range(2):
                    tk = ds((t8 % 4) * 512 + half * 256, 256)
                    o = ps[:, ts(half, 256)]
                    n = 0
                    for wsb, xsb in ((wh, xh_sb), (wh, xr_sb), (wr, xh_sb)):
                        for a in range(HC):
                            nc.tensor.matmul(
                                o,
                                wsb[:, a, :, :],
                                xsb[:, a, :, tk],
                                start=(n == 0),
                                stop=(n == 3 * HC - 1),
                                perf_mode=DR,
                            )
                            n += 1
                if which == "q":
                    nc.vector.tensor_scalar(
                        dst[:, ts(t8, 512)], ps, 1.0 / 512.0, bq_sb, MULT, ADD
                    )
                else:
                    nc.vector.tensor_scalar_mul(dst[:, ts(t8, 512)], ps, 1.0 / 512.0)

            def v_tile(t32):
                """V for one 128-token tile, direct [tok, col] layout."""
                wh, wr = w_sbs["whv"], w_sbs["wrv"]
                ps = psp.tile([P, P], f32, tag="ctx", bufs=2, name="vps")
                tk = ds((t32 % 16) * P, P)
                n = 0
                for wsb, xsb in ((wh, xh_sb), (wh, xr_sb), (wr, xh_sb)):
                    for a in range(HC):
                        nc.tensor.matmul(
                            ps,
                            xsb[:, a, :, tk],
                            wsb[:, a, :, :],
                            start=(n == 0),
                            stop=(n == 3 * HC - 1),
                            perf_mode=DR,
                        )
                        n += 1
                dst = vp_sb[:, t32, :, 0:D]
                nc.vector.tensor_scalar_mul(
                    dst, ps.rearrange("p (h d) -> p h d", h=2), 1.0 / 512.0
                )

            # PT ring: 2 heads x 32 slots x [128, 1024] fp16
            RING = 32
            pt_all = ptpool.tile([P, 2, RING, 1024], f16, tag="pt")

            ot_tiles = {}

            def pv_qc(unit, head, qc):
                """P@V for one 128-q chunk: ctx[q,65] += PT[k,qc].T @ V'."""
                b = unit // 2
                ctx = psp.tile([P, D + 1], f32, tag="ctx", bufs=2, name="ctx")
                for kt in range(16):
                    nc.tensor.matmul(
                        ctx,
                        pt_all[:, head, (unit * 16 + kt) % RING, ts(qc, P)],
                        vp_sb[:, b * 16 + kt, head, 0 : D + 1],
                        start=(kt == 0),
                        stop=(kt == 15),
                    )
                return ctx

            def pv_norm(ctx, unit, head, qc):
                """Normalize one ctx chunk into the (unit, head) staging tile;
                DMA the full 1024 q rows out after the last chunk."""
                qbase = (unit // 2) * S + (unit % 2) * 1024
                key = (unit, head)
                if key not in ot_tiles:
                    ot_tiles[key] = otpool.tile([P, 8, D], f32, tag="ot", name="ot")
                ot = ot_tiles[key]
                rc = smallpool.tile([P, 1], f32, tag="rc")
                nc.vector.reciprocal(rc, ctx[:, D : D + 1])
                nc.vector.tensor_scalar_mul(ot[:, qc, :], ctx[:, 0:D], rc)
                if qc in (3, 7):
                    half = qc // 4
                    dst = out[
                        ds(qbase + half * 512, 512), ds(D * head, D)
                    ].rearrange("(qc p) d -> p qc d", p=P)
                    nc.sync.dma_start(dst, ot[:, ds(half * 4, 4), :])

            def pv_full(unit, head, qc):
                ctx = pv_qc(unit, head, qc)
                pv_norm(ctx, unit, head, qc)

            # Deferred-work queue: (cost, fn, deadline). Deadline (u, kt)
            # means the item MUST be emitted before (u, kt)'s scores/exp.
            work_q = deque()

            def q_proj(t8, which, dl):
                work_q.append((1.28, lambda: proj_group(t8, which), dl))

            def q_v(tiles, dl):
                for t in tiles:
                    work_q.append((0.32, lambda tt=t: v_tile(tt), dl))

            def q_pv(unit, head, qcs, dl):
                for qc in qcs:
                    work_q.append(
                        (0.43, lambda h=head, q=qc: pv_full(unit, h, q), dl)
                    )

            NEVER = (9, 0)

            def push_unit_work(unit):
                if unit == 0:
                    q_proj(1, "k", (0, 4))  # k1
                    q_v(range(0, 4), (1, 0))
                    q_proj(2, "q", (1, 0))  # q2 (unit 1 scores)
                    q_proj(2, "k", (0, 8))  # k2
                    q_v(range(4, 8), (1, 0))
                    q_proj(3, "q", (1, 0))  # q3
                    q_proj(3, "k", (0, 12))  # k3
                    q_v(range(8, 16), (1, 0))
                elif unit == 1:
                    q_pv(0, 0, range(0, 4), (2, 0))
                    q_proj(4, "k", (2, 0))  # k4
                    q_pv(0, 0, range(4, 8), (2, 0))
                    q_proj(4, "q", (2, 0))  # q4
                    q_pv(0, 1, range(0, 4), (2, 0))
                    q_proj(5, "q", (2, 0))  # q5
                    q_pv(0, 1, range(4, 8), (2, 0))
                    q_proj(5, "k", (2, 4))  # k5
                    q_v(range(16, 20), (3, 0))
                elif unit == 2:
                    q_proj(6, "k", (2, 8))  # k6
                    q_pv(1, 0, range(0, 4), (3, 0))
                    q_proj(7, "k", (2, 12))  # k7
                    q_pv(1, 0, range(4, 8), (3, 0))
                    q_proj(6, "q", (3, 0))  # q6
                    q_pv(1, 1, range(0, 4), (3, 0))
                    q_proj(7, "q", (3, 0))  # q7
                    q_pv(1, 1, range(4, 8), (3, 0))
                    q_v(range(20, 32), (3, 0))
                elif unit == 3:
                    q_pv(2, 0, range(8), NEVER)
                    q_pv(2, 1, range(8), NEVER)

            # ---- batch 0 essentials: just enough for unit 0's scores ----
            proj_group(0, "q")
            proj_group(1, "q")
            proj_group(0, "k")

            for unit in range(4):
                b, qh = unit // 2, unit % 2
                base = b * S
                qbase = base + qh * 1024
                if unit == 1:
                    # drain every batch-0 consumer of x first (emission
                    # order is semantic order), then reload X^T with batch 1
                    while work_q and work_q[0][2] <= (1, 0):
                        work_q.popleft()[1]()
                    for quarter in range(4):
                        nc.sync.dma_start(
                            xh_sb[:, :, :, ts(quarter, 512)],
                            xhp[:, :, :, ds(S + quarter * 512, 512)],
                        )
                        nc.sync.dma_start(
                            xr_sb[:, :, :, ts(quarter, 512)],
                            xrp[:, :, :, ds(S + quarter * 512, 512)],
                        )
                push_unit_work(unit)
                credit = 2.0
                for kt in range(16):
                    while work_q and work_q[0][2] <= (unit, kt):
                        _, fn, _ = work_q.popleft()
                        fn()
                    sts = []
                    for head in range(2):
                        st = psp.tile(
                            [P, 1024], f32, tag="st", bufs=2, name=f"st{head}"
                        )
                        sts.append(st)
                    for j in range(2):
                        for head in range(2):
                            hb = D * head
                            nc.tensor.matmul(
                                sts[head][:, ts(j, 512)],
                                kt_sb[ds(hb, D), ds(base + kt * P, P)],
                                qt_sb[ds(hb, D), ds(qbase + j * 512, 512)],
                                start=True,
                                stop=True,
                            )
                    slot = (unit * 16 + kt) % RING
                    for head in range(2):
                        dstp = pt_all[:, head, slot, :]
                        dve_js = [
                            j for j in range(2) if (head, j) in DVE_JH[unit]
                        ]
                        if not dve_js:
                            nc.scalar.activation(dstp, sts[head], Exp, scale=0.125)
                        else:
                            for j in range(2):
                                dst = dstp[:, ts(j, 512)]
                                src = sts[head][:, ts(j, 512)]
                                if j in dve_js:
                                    nc.vector.tensor_scalar(
                                        dst.bitcast(u16), src, A16, B16, MULT, ADD
                                    )
                                else:
                                    nc.scalar.activation(dst, src, Exp, scale=0.125)
                    # deferred work drained under the kt shadow, paced
                    credit = min(credit + 1.3, 8.0)
                    while work_q and work_q[0][0] <= credit:
                        cost, fn, _ = work_q.popleft()
                        credit -= cost
                        fn()
            while work_q:
                work_q.popleft()[1]()
            # unit 3's P@V has no later exp shadow: straight-line tail,
            # heads interleaved so norms/DMAs overlap remaining matmuls
            for qc in range(8):
                for head in range(2):
                    pv_full(3, head, qc)

    nc.compile()
    return nc


def get_nc():
    if "nc" not in _CACHED:
        _CACHED["nc"] = _build()
    return _CACHED["nc"]


def _wsplit(Wslice):
    """Two-term fp8 split of a weight column slice at common scale 512.

    Returns (wh16, wr) fp8 arrays with W ~= (wh16 + wr) / 512, where
    wh16 = fp8(W*32)*16 exactly (exponent shift) and wr = fp8(512*(W - fp8(W*32)/32)).
    """
    w32 = (Wslice * 32.0).astype(F8)
    wh16 = (w32.astype(np.float32) * 16.0).astype(F8)
    wr = ((Wslice - w32.astype(np.float32) / 32.0) * 512.0).astype(F8)
    return wh16, wr


def kernel(hidden_states, Wq, bq, Wk, bk, Wv, bv):
    from concourse.bass_utils import run_bass_kernel_spmd

    nc = get_nc()

    x2 = np.asarray(hidden_states, dtype=np.float32).reshape(T, HID)
    xt = np.ascontiguousarray(x2.T)
    xh = xt.astype(F8)
    xr = (xt - xh.astype(np.float32)).astype(F8)

    Wq = np.asarray(Wq, np.float32)
    Wk = np.asarray(Wk, np.float32)
    Wv = np.asarray(Wv, np.float32)
    bqf = np.asarray(bq, np.float32)
    bvf = np.asarray(bv, np.float32)

    in_maps = []
    for c in range(N_CORES):
        sl = slice(P * c, P * (c + 1))
        whq, wrq = _wsplit(Wq[:, sl])
        whk, wrk = _wsplit(Wk[:, sl])
        whv, wrv = _wsplit(Wv[:, sl])
        in_maps.append(
            {
                "xh": xh,
                "xr": xr,
                "whq": np.ascontiguousarray(whq),
                "wrq": np.ascontiguousarray(wrq),
                "whk": np.ascontiguousarray(whk),
                "wrk": np.ascontiguousarray(wrk),
                "whv": np.ascontiguousarray(whv),
                "wrv": np.ascontiguousarray(wrv),
                "bq": np.ascontiguousarray(bqf[sl][:, None]),
            }
        )

    res = run_bass_kernel_spmd(nc, in_maps, list(range(N_CORES)))

    full = np.empty((T, HID), dtype=np.float32)
    for c in range(N_CORES):
        full[:, P * c : P * (c + 1)] = res.results[c]["out"]
    full += bvf[None, :]
    return full.reshape(B, S, HID)


# revision 4
# speedup vs baseline: 1.1023x; 1.1023x over previous
"""BERT attention (QKV proj + SDPA) sharded over 8 trn2 NeuronCores by head.

Problem: hidden_states [2, 2048, 1024], 16 heads x 64 dim, fp32.
Sharding: 2 heads per core (tensor-parallel on Q/K/V weight columns).

Math shortcuts (exact):
  - bk drops out entirely (softmax is invariant to per-row score shifts).
  - bv is added on the HOST after the gather (ctx = softmax(P)@(V+bv) =
    softmax(P)@V + bv).

Per-core device kernel:
  inputs:  xh/xr [1024, 4096] fp8e4m3  X^T two-term split (x = xh + xr)
           wh*/wr* [1024, 128] fp8     weight two-term split at common
                                       scale 512: W ~ (wh*16 + wr)/512
           bq [128, 1] f32             q bias slice
  output:  out [4096, 128] f32         context for this core's 2 heads

Dataflow, 8 units of 512 q-rows (unit = batch*4 + quarter), 16 kt each:
  1. Q/K projections as 3-pass fp8 DoubleRow matmuls (contraction 256/instr,
     0.5 cyc/col): psum = xh@wh16 + xr@wh16 + xh@wr (all scale-512), then
     DVE copy psum*(1/512)(+bq) -> fp16 qt/kt  (k needs no bias).
  2. V computed DIRECTLY in [token, dim] layout (stationary = X^T tile,
     moving = wv): no PE transposes. DVE copies psum*(1/512) into the
     ones-augmented V' tile [128k, kt, head, 65].
  3. Scores TRANSPOSED fp16: ST[k, q=512] f32 psum per head; st tiles are
     one PSUM bank with bufs=4 so the score->exp chain has 2 kt of depth.
  4. exp: ACT native Exp (scale 1/8); late units offload one head per unit
     to DVE via the Schraudolph u16-bits trick (bits = round(s*1024/(8 ln2)
     + 15301.5) bitcast fp16) -- late units are PE-light, and flavor is
     UNIFORM along k per q-row so each flavor's bias cancels in the
     softmax denominator (ones column).
  5. P@V with q stationary: ctx[q=128, d|sum 65] = sum_kt PT[k,qc].T@V'[k,65];
     DVE reciprocal of the sums column + tensor_scalar multiply, DMA out
     per 512 q-rows. pv(u) runs as fillers inside unit u+1's kt loop.

Fillers (projections, V tiles, deferred P@V) are emitted inside the kt loop
paced by a credit system. PSUM: scores 4x1 banks, proj 2x1, ctx/v 2x1.
"""

import numpy as np
import ml_dtypes

B, S, HID = 2, 2048, 1024
T = B * S
N_CORES = 8
P = 128
D = 64
HC = 4  # hidden chunks of 256 (DoubleRow pairs of 128)

F16 = np.float16
F8 = ml_dtypes.float8_e4m3

_CACHED = {}

# units whose given head's exp runs on DVE (Schraudolph); ~25% of rows.
DVE_H = {4: (1,), 5: (0,), 6: (1,), 7: (0,)}
A16 = 0.125 * 1024.0 / np.log(2.0)
B16 = 15.0 * 1024.0 - 58.5  # centered Schraudolph bias


def _build():
    from collections import deque

    import concourse.bass as bass  # noqa: F401
    from concourse import bacc
    import concourse.tile as tile
    import concourse.mybir as mybir
    from concourse.bass import ts, ds

    f8 = mybir.dt.float8e4
    f16 = mybir.dt.float16
    f32 = mybir.dt.float32
    u16 = mybir.dt.uint16
    Exp = mybir.ActivationFunctionType.Exp
    DR = mybir.MatmulPerfMode.DoubleRow
    MULT = mybir.AluOpType.mult
    ADD = mybir.AluOpType.add

    nc = bacc.Bacc(trn_type="TRN2", target_bir_lowering=False, debug=False)

    xh = nc.dram_tensor("xh", [HID, T], f8, kind="ExternalInput").ap()
    xr = nc.dram_tensor("xr", [HID, T], f8, kind="ExternalInput").ap()
    wts = {}
    for nm in ("whq", "wrq", "whk", "wrk", "whv", "wrv"):
        wts[nm] = nc.dram_tensor(nm, [HID, P], f8, kind="ExternalInput").ap()
    bq = nc.dram_tensor("bq", [P, 1], f32, kind="ExternalInput").ap()
    out = nc.dram_tensor("out", [T, P], f32, kind="ExternalOutput").ap()

    with tile.TileContext(nc) as tc:
        with (
            tc.tile_pool(name="const", bufs=1) as cpool,
            tc.tile_pool(name="xtp", bufs=1) as xtpool,
            tc.tile_pool(name="qkv", bufs=1) as qkvpool,
            tc.tile_pool(name="pt", bufs=1) as ptpool,
            tc.tile_pool(name="small", bufs=4) as smallpool,
            tc.tile_pool(name="ot", bufs=3) as otpool,
            tc.tile_pool(name="ps", bufs=2, space="PSUM") as psp,
        ):
            # X^T half-buffer (one batch), two fp8 terms; hid = a*256+i*128+p
            xh_sb = xtpool.tile([P, HC, 2, S], f8, tag="xh")
            xr_sb = xtpool.tile([P, HC, 2, S], f8, tag="xr")
            xhp = xh.rearrange("(a i p) t -> p a i t", p=P, i=2)
            xrp = xr.rearrange("(a i p) t -> p a i t", p=P, i=2)
            w_sbs = {}
            bq_sb = cpool.tile([P, 1], f32, tag="bq")
            for nm in wts:
                w_sbs[nm] = cpool.tile([P, HC, 2, P], f8, tag=nm, name=nm)
            # DMA arrival order matched to first-consumption order
            nc.sync.dma_start(xh_sb[:, :, :, 0:512], xhp[:, :, :, 0:512])
            nc.sync.dma_start(bq_sb, bq)
            nc.sync.dma_start(w_sbs["whq"], wts["whq"].rearrange("(a i p) c -> p a i c", p=P, i=2))
            nc.sync.dma_start(w_sbs["wrq"], wts["wrq"].rearrange("(a i p) c -> p a i c", p=P, i=2))
            nc.sync.dma_start(xr_sb[:, :, :, 0:512], xrp[:, :, :, 0:512])
            nc.sync.dma_start(w_sbs["whk"], wts["whk"].rearrange("(a i p) c -> p a i c", p=P, i=2))
            nc.sync.dma_start(w_sbs["wrk"], wts["wrk"].rearrange("(a i p) c -> p a i c", p=P, i=2))
            nc.sync.dma_start(xh_sb[:, :, :, ts(1, 512)], xhp[:, :, :, ts(1, 512)])
            nc.sync.dma_start(xr_sb[:, :, :, ts(1, 512)], xrp[:, :, :, ts(1, 512)])
            nc.sync.dma_start(w_sbs["whv"], wts["whv"].rearrange("(a i p) c -> p a i c", p=P, i=2))
            nc.sync.dma_start(w_sbs["wrv"], wts["wrv"].rearrange("(a i p) c -> p a i c", p=P, i=2))
            for quarter in range(2, 4):
                nc.sync.dma_start(xh_sb[:, :, :, ts(quarter, 512)], xhp[:, :, :, ts(quarter, 512)])
                nc.sync.dma_start(xr_sb[:, :, :, ts(quarter, 512)], xrp[:, :, :, ts(quarter, 512)])

            # zero operand for PE warm-up (values unused)
            garb = cpool.tile([P, 512], f16, tag="garb")
            nc.gpsimd.memset(garb, 0.0)

            qt_sb = qkvpool.tile([P, T], f16, tag="qt")
            kt_sb = qkvpool.tile([P, T], f16, tag="kt")
            # V' [k-part, ktile(32), head(2), 66]; col 64 = ones (row sums)
            vp_sb = qkvpool.tile([P, T // P, 2, D + 2], f16, tag="vp")
            nc.gpsimd.memset(vp_sb[:, :, :, D : D + 1], 1.0)

            # PE warm-up while the first DMAs land: long enough that the
            # p-state ramp completes AND the PE never idles before the first
            # projection.
            wu = psp.tile([P, 512], f32, tag="pj", bufs=2, name="wups")
            NWU = 17
            for i in range(NWU):
                nc.tensor.matmul(
                    wu, garb[:, 0:P], garb, start=(i == 0), stop=(i == NWU - 1)
                )
            nc.vector.tensor_copy(kt_sb[:, 0:512], wu)  # dummy drain (overwritten)

            def proj_group(t8, which):
                """Project 512 tokens (chunk t8) for q or k: 3-pass fp8 DR."""
                wh, wr = w_sbs["wh" + which], w_sbs["wr" + which]
                dst = qt_sb if which == "q" else kt_sb
                ps = psp.tile([P, 512], f32, tag="pj", bufs=2, name="projps")
                for half in range(2):
                    tk = ds((t8 % 4) * 512 + half * 256, 256)
                    o = ps[:, ts(half, 256)]
                    n = 0
                    for wsb, xsb in ((wh, xh_sb), (wh, xr_sb), (wr, xh_sb)):
                        for a in range(HC):
                            nc.tensor.matmul(
                                o,
                                wsb[:, a, :, :],
                                xsb[:, a, :, tk],
                                start=(n == 0),
                                stop=(n == 3 * HC - 1),
                                perf_mode=DR,
                            )
                            n += 1
                if which == "q":
                    nc.vector.tensor_scalar(
                        dst[:, ts(t8, 512)], ps, 1.0 / 512.0, bq_sb, MULT, ADD
                    )
                else:
                    nc.vector.tensor_scalar_mul(dst[:, ts(t8, 512)], ps, 1.0 / 512.0)

            def v_tile(t32):
                """V for one 128-token tile, direct [tok, col] layout."""
                wh, wr = w_sbs["whv"], w_sbs["wrv"]
                ps = psp.tile([P, P], f32, tag="ctx", bufs=2, name="vps")
                tk = ds((t32 % 16) * P, P)
                n = 0
                for wsb, xsb in ((wh, xh_sb), (wh, xr_sb), (wr, xh_sb)):
                    for a in range(HC):
                        nc.tensor.matmul(
                            ps,
                            xsb[:, a, :, tk],
                            wsb[:, a, :, :],
                            start=(n == 0),
                            stop=(n == 3 * HC - 1),
                            perf_mode=DR,
                        )
                        n += 1
                dst = vp_sb[:, t32, :, 0:D]
                nc.vector.tensor_scalar_mul(
                    dst, ps.rearrange("p (h d) -> p h d", h=2), 1.0 / 512.0
                )

            # PT ring: 2 heads x 32 slots x [128, 512] fp16 (2 units deep)
            RING = 32
            pt_all = ptpool.tile([P, 2, RING, 512], f16, tag="pt")

            ot_tiles = {}

            def pv_qc(unit, head, qc):
                """P@V for one 128-q chunk: ctx[q,65] += PT[k,qc].T @ V'."""
                b = unit // 4
                ctx = psp.tile([P, D + 1], f32, tag="ctx", bufs=2, name="ctx")
                for kt in range(16):
                    nc.tensor.matmul(
                        ctx,
                        pt_all[:, head, (unit * 16 + kt) % RING, ts(qc, P)],
                        vp_sb[:, b * 16 + kt, head, 0 : D + 1],
                        start=(kt == 0),
                        stop=(kt == 15),
                    )
                return ctx

            def pv_norm(ctx, unit, head, qc):
                """Normalize one ctx chunk into the (unit, head) staging tile;
                DMA the 512 q rows out after the last chunk."""
                qbase = (unit // 4) * S + (unit % 4) * 512
                key = (unit, head)
                if key not in ot_tiles:
                    ot_tiles[key] = otpool.tile([P, 4, D], f32, tag="ot", name="ot")
                ot = ot_tiles[key]
                rc = smallpool.tile([P, 1], f32, tag="rc")
                nc.vector.reciprocal(rc, ctx[:, D : D + 1])
                nc.vector.tensor_scalar_mul(ot[:, qc, :], ctx[:, 0:D], rc)
                if qc == 3:
                    dst = out[ds(qbase, 512), ds(D * head, D)].rearrange(
                        "(qc p) d -> p qc d", p=P
                    )
                    nc.sync.dma_start(dst, ot)
                    del ot_tiles[key]

            def pv_full(unit, head, qc):
                ctx = pv_qc(unit, head, qc)
                pv_norm(ctx, unit, head, qc)

            # Deferred-work queue: (cost, fn, deadline). Deadline (u, kt)
            # means the item MUST be emitted before (u, kt)'s scores/exp.
            work_q = deque()

            def q_proj(t8, which, dl):
                work_q.append((1.28, lambda: proj_group(t8, which), dl))

            def q_v(tiles, dl):
                for t in tiles:
                    work_q.append((0.32, lambda tt=t: v_tile(tt), dl))

            def q_pv(unit, head, qcs, dl):
                for qc in qcs:
                    work_q.append(
                        (0.43, lambda h=head, q=qc: pv_full(unit, h, q), dl)
                    )

            def push_unit_work(unit):
                if unit == 0:
                    q_proj(1, "k", (0, 4))  # k1
                    q_proj(1, "q", (1, 0))  # q1
                    q_proj(2, "k", (0, 8))  # k2
                    q_v(range(0, 4), (1, 0))
                    q_proj(3, "k", (0, 12))  # k3
                    q_v(range(4, 8), (1, 0))
                elif unit == 1:
                    q_v(range(8, 16), (1, 12))
                    q_pv(0, 0, range(4), (2, 0))
                    q_proj(2, "q", (2, 0))  # q2
                    q_pv(0, 1, range(4), (2, 0))
                    q_proj(3, "q", (2, 0))  # q3
                elif unit == 2:
                    q_pv(1, 0, range(4), (3, 0))
                    q_proj(4, "k", (4, 0))  # k4
                    q_pv(1, 1, range(4), (3, 0))
                    q_proj(5, "k", (4, 4))  # k5
                    q_v(range(16, 20), (5, 0))
                elif unit == 3:
                    q_pv(2, 0, range(4), (4, 0))
                    q_proj(6, "k", (4, 8))  # k6
                    q_pv(2, 1, range(4), (4, 0))
                    q_proj(7, "k", (4, 12))  # k7
                    q_proj(4, "q", (4, 0))  # q4
                    q_v(range(20, 24), (5, 0))
                elif unit == 4:
                    q_pv(3, 0, range(4), (5, 0))
                    q_proj(5, "q", (5, 0))  # q5
                    q_pv(3, 1, range(4), (5, 0))
                    q_v(range(24, 32), (5, 0))
                elif unit == 5:
                    q_pv(4, 0, range(4), (6, 0))
                    q_proj(6, "q", (6, 0))  # q6
                    q_pv(4, 1, range(4), (6, 0))
                    q_proj(7, "q", (7, 0))  # q7
                elif unit == 6:
                    q_pv(5, 0, range(4), (7, 0))
                    q_pv(5, 1, range(4), (7, 0))
                elif unit == 7:
                    q_pv(6, 0, range(4), (8, 0))
                    q_pv(6, 1, range(4), (8, 0))

            # ---- batch 0 essentials: just enough for unit 0's scores ----
            proj_group(0, "q")
            proj_group(0, "k")

            for unit in range(8):
                b = unit // 4
                base = b * S
                qbase = base + (unit % 4) * 512
                if unit == 2:
                    # drain every batch-0 consumer of x first (emission
                    # order is semantic order), then reload X^T with batch 1
                    while work_q and work_q[0][2] <= (2, 0):
                        work_q.popleft()[1]()
                    for quarter in range(4):
                        nc.sync.dma_start(
                            xh_sb[:, :, :, ts(quarter, 512)],
                            xhp[:, :, :, ds(S + quarter * 512, 512)],
                        )
                        nc.sync.dma_start(
                            xr_sb[:, :, :, ts(quarter, 512)],
                            xrp[:, :, :, ds(S + quarter * 512, 512)],
                        )
                push_unit_work(unit)
                credit = 2.0
                for kt in range(16):
                    while work_q and work_q[0][2] <= (unit, kt):
                        _, fn, _ = work_q.popleft()
                        fn()
                    slot = (unit * 16 + kt) % RING
                    for head in range(2):
                        hb = D * head
                        st = psp.tile([P, 512], f32, tag="st", bufs=4, name="st")
                        nc.tensor.matmul(
                            st,
                            kt_sb[ds(hb, D), ds(base + kt * P, P)],
                            qt_sb[ds(hb, D), ds(qbase, 512)],
                            start=True,
                            stop=True,
                        )
                        dst = pt_all[:, head, slot, :]
                        if head in DVE_H.get(unit, ()):
                            nc.vector.tensor_scalar(
                                dst.bitcast(u16), st, A16, B16, MULT, ADD
                            )
                        else:
                            nc.scalar.activation(dst, st, Exp, scale=0.125)
                    # deferred work drained under the kt shadow, paced
                    credit = min(credit + 0.65, 6.0)
                    while work_q and work_q[0][0] <= credit:
                        cost, fn, _ = work_q.popleft()
                        credit -= cost
                        fn()
            while work_q:
                work_q.popleft()[1]()
            # unit 7's P@V has no later exp shadow: straight-line tail,
            # heads interleaved so norms/DMAs overlap remaining matmuls
            for qc in range(4):
                for head in range(2):
                    pv_full(7, head, qc)

    nc.compile()
    return nc


def get_nc():
    if "nc" not in _CACHED:
        _CACHED["nc"] = _build()
    return _CACHED["nc"]


def _wsplit(Wslice):
    """Two-term fp8 split of a weight column slice at common scale 512.

    Returns (wh16, wr) fp8 arrays with W ~= (wh16 + wr) / 512, where
    wh16 = fp8(W*32)*16 exactly (exponent shift) and wr = fp8(512*(W - fp8(W*32)/32)).
    """
    w32 = (Wslice * 32.0).astype(F8)
    wh16 = (w32.astype(np.float32) * 16.0).astype(F8)
    wr = ((Wslice - w32.astype(np.float32) / 32.0) * 512.0).astype(F8)
    return wh16, wr


def kernel(hidden_states, Wq, bq, Wk, bk, Wv, bv):
    from concourse.bass_utils import run_bass_kernel_spmd

    nc = get_nc()

    x2 = np.asarray(hidden_states, dtype=np.float32).reshape(T, HID)
    xt = np.ascontiguousarray(x2.T)
    xh = xt.astype(F8)
    xr = (xt - xh.astype(np.float32)).astype(F8)

    Wq = np.asarray(Wq, np.float32)
    Wk = np.asarray(Wk, np.float32)
    Wv = np.asarray(Wv, np.float32)
    bqf = np.asarray(bq, np.float32)
    bvf = np.asarray(bv, np.float32)

    in_maps = []
    for c in range(N_CORES):
        sl = slice(P * c, P * (c + 1))
        whq, wrq = _wsplit(Wq[:, sl])
        whk, wrk = _wsplit(Wk[:, sl])
        whv, wrv = _wsplit(Wv[:, sl])
        in_maps.append(
            {
                "xh": xh,
                "xr": xr,
                "whq": np.ascontiguousarray(whq),
                "wrq": np.ascontiguousarray(wrq),
                "whk": np.ascontiguousarray(whk),
                "wrk": np.ascontiguousarray(wrk),
                "whv": np.ascontiguousarray(whv),
                "wrv": np.ascontiguousarray(wrv),
                "bq": np.ascontiguousarray(bqf[sl][:, None]),
            }
        )

    res = run_bass_kernel_spmd(nc, in_maps, list(range(N_CORES)))

    full = np.empty((T, HID), dtype=np.float32)
    for c in range(N_CORES):
        full[:, P * c : P * (c + 1)] = res.results[c]["out"]
    full += bvf[None, :]
    return full.reshape(B, S, HID)


# revision 8
# speedup vs baseline: 1.2270x; 1.1132x over previous
"""BERT attention (QKV proj + SDPA) sharded over 8 trn2 NeuronCores by head.

Problem: hidden_states [2, 2048, 1024], 16 heads x 64 dim, fp32.
Sharding: 2 heads per core (tensor-parallel on Q/K/V weight columns).

Math shortcuts (exact):
  - bk drops out entirely (softmax is invariant to per-row score shifts).
  - bv is added on the HOST after the gather (ctx = softmax(P)@(V+bv) =
    softmax(P)@V + bv).

Per-core device kernel:
  inputs:  xh/xr [1024, 4096] fp8e4m3  X^T two-term split (x = xh + xr)
           wh*/wr* [1024, 128] fp8     weight two-term split at common
                                       scale 512: W ~ (wh*16 + wr)/512
           bq [128, 1] f32             q bias slice
  output:  out [4096, 128] f32         context for this core's 2 heads

Dataflow, 8 units of 512 q-rows (unit = batch*4 + quarter), 16 kt each:
  1. Q/K projections as 3-pass fp8 DoubleRow matmuls (contraction 256/instr,
     0.5 cyc/col): psum = xh@wh16 + xr@wh16 + xh@wr (all scale-512), then
     DVE copy psum*(1/512)(+bq) -> fp16 qt/kt  (k needs no bias).
  2. V computed DIRECTLY in [token, dim] layout (stationary = X^T tile,
     moving = wv): no PE transposes. DVE copies psum*(1/512) into the
     ones-augmented V' tile [128k, kt, head, 65].
  3. Scores TRANSPOSED fp16: ST[k, q=512] f32 psum per head; st tiles are
     one PSUM bank with bufs=4 so the score->exp chain has 2 kt of depth.
  4. exp: ACT native Exp (scale 1/8); late units offload one head per unit
     to DVE via the Schraudolph u16-bits trick (bits = round(s*1024/(8 ln2)
     + 15301.5) bitcast fp16) -- late units are PE-light, and flavor is
     UNIFORM along k per q-row so each flavor's bias cancels in the
     softmax denominator (ones column).
  5. P@V with q stationary: ctx[q=128, d|sum 65] = sum_kt PT[k,qc].T@V'[k,65];
     DVE reciprocal of the sums column + tensor_scalar multiply, DMA out
     per 512 q-rows. pv(u) runs as fillers inside unit u+1's kt loop.

Fillers (projections, V tiles, deferred P@V) are emitted inside the kt loop
paced by a credit system. PSUM: scores 4x1 banks, proj 2x1, ctx/v 2x1.
"""

import numpy as np
import ml_dtypes

B, S, HID = 2, 2048, 1024
T = B * S
N_CORES = 8
P = 128
D = 64
HC = 4  # hidden chunks of 256 (DoubleRow pairs of 128)

F16 = np.float16
F8 = ml_dtypes.float8_e4m3

_CACHED = {}

# units whose given head's exp runs on DVE (Schraudolph); 50% of rows, one
# head per unit so ACT and DVE each carry one exp op per kt.
DVE_H = {u: (u % 2,) for u in range(8)}
A16 = 0.125 * 1024.0 / np.log(2.0)
B16 = 15.0 * 1024.0 - 58.5  # centered Schraudolph bias


def _build():
    from collections import deque

    import concourse.bass as bass  # noqa: F401
    from concourse import bacc
    import concourse.tile as tile
    import concourse.mybir as mybir
    from concourse.bass import ts, ds

    f8 = mybir.dt.float8e4
    f16 = mybir.dt.float16
    f32 = mybir.dt.float32
    u16 = mybir.dt.uint16
    Exp = mybir.ActivationFunctionType.Exp
    Copy = mybir.ActivationFunctionType.Copy
    Ident = mybir.ActivationFunctionType.Identity
    DR = mybir.MatmulPerfMode.DoubleRow
    MULT = mybir.AluOpType.mult
    ADD = mybir.AluOpType.add

    nc = bacc.Bacc(trn_type="TRN2", target_bir_lowering=False, debug=False)

    xh = nc.dram_tensor("xh", [HID, T], f8, kind="ExternalInput").ap()
    xr = nc.dram_tensor("xr", [HID, T], f8, kind="ExternalInput").ap()
    wts = {}
    for nm in ("whq", "wrq", "whk", "wrk", "whv", "wrv"):
        wts[nm] = nc.dram_tensor(nm, [HID, P], f8, kind="ExternalInput").ap()
    bq = nc.dram_tensor("bq", [P, 1], f32, kind="ExternalInput").ap()
    out = nc.dram_tensor("out", [T, P], f32, kind="ExternalOutput").ap()

    with tile.TileContext(nc) as tc:
        with (
            tc.tile_pool(name="const", bufs=1) as cpool,
            tc.tile_pool(name="xtp", bufs=1) as xtpool,
            tc.tile_pool(name="qkv", bufs=1) as qkvpool,
            tc.tile_pool(name="pt", bufs=1) as ptpool,
            tc.tile_pool(name="small", bufs=4) as smallpool,
            tc.tile_pool(name="ot", bufs=3) as otpool,
            tc.tile_pool(name="ps", bufs=2, space="PSUM") as psp,
        ):
            # X^T half-buffer (one batch), two fp8 terms; hid = a*256+i*128+p
            xh_sb = xtpool.tile([P, HC, 2, S], f8, tag="xh")
            xr_sb = xtpool.tile([P, HC, 2, S], f8, tag="xr")
            xhp = xh.rearrange("(a i p) t -> p a i t", p=P, i=2)
            xrp = xr.rearrange("(a i p) t -> p a i t", p=P, i=2)
            w_sbs = {}
            bq_sb = cpool.tile([P, 1], f32, tag="bq")
            for nm in wts:
                w_sbs[nm] = cpool.tile([P, HC, 2, P], f8, tag=nm, name=nm)
            # DMA arrival order matched to first-consumption order
            nc.sync.dma_start(xh_sb[:, :, :, 0:512], xhp[:, :, :, 0:512])
            nc.sync.dma_start(bq_sb, bq)
            nc.sync.dma_start(w_sbs["whq"], wts["whq"].rearrange("(a i p) c -> p a i c", p=P, i=2))
            nc.sync.dma_start(w_sbs["wrq"], wts["wrq"].rearrange("(a i p) c -> p a i c", p=P, i=2))
            nc.sync.dma_start(xr_sb[:, :, :, 0:512], xrp[:, :, :, 0:512])
            nc.sync.dma_start(w_sbs["whk"], wts["whk"].rearrange("(a i p) c -> p a i c", p=P, i=2))
            nc.sync.dma_start(w_sbs["wrk"], wts["wrk"].rearrange("(a i p) c -> p a i c", p=P, i=2))
            nc.sync.dma_start(xh_sb[:, :, :, ts(1, 512)], xhp[:, :, :, ts(1, 512)])
            nc.sync.dma_start(xr_sb[:, :, :, ts(1, 512)], xrp[:, :, :, ts(1, 512)])
            nc.sync.dma_start(w_sbs["whv"], wts["whv"].rearrange("(a i p) c -> p a i c", p=P, i=2))
            nc.sync.dma_start(w_sbs["wrv"], wts["wrv"].rearrange("(a i p) c -> p a i c", p=P, i=2))
            for quarter in range(2, 4):
                nc.sync.dma_start(xh_sb[:, :, :, ts(quarter, 512)], xhp[:, :, :, ts(quarter, 512)])
                nc.sync.dma_start(xr_sb[:, :, :, ts(quarter, 512)], xrp[:, :, :, ts(quarter, 512)])

            # zero operand for PE warm-up (values unused)
            garb = cpool.tile([P, 512], f16, tag="garb")
            nc.gpsimd.memset(garb, 0.0)

            qt_sb = qkvpool.tile([P, T], f16, tag="qt")
            kt_sb = qkvpool.tile([P, T], f16, tag="kt")
            # V' [k-part, ktile(32), head(2), 66]; col 64 = ones (row sums)
            vp_sb = qkvpool.tile([P, T // P, 2, D + 2], f16, tag="vp")
            nc.gpsimd.memset(vp_sb[:, :, :, D : D + 1], 1.0)

            # PE warm-up while the first DMAs land: long enough that the
            # p-state ramp completes AND the PE never idles before the first
            # projection.
            wu = psp.tile([P, 512], f32, tag="pj", bufs=2, name="wups")
            NWU = 17
            for i in range(NWU):
                nc.tensor.matmul(
                    wu, garb[:, 0:P], garb, start=(i == 0), stop=(i == NWU - 1)
                )
            nc.vector.tensor_copy(kt_sb[:, 0:512], wu)  # dummy drain (overwritten)

            def proj_group(t8, which):
                """Project 512 tokens (chunk t8) for q or k: 3-pass fp8 DR."""
                wh, wr = w_sbs["wh" + which], w_sbs["wr" + which]
                dst = qt_sb if which == "q" else kt_sb
                ps = psp.tile([P, 512], f32, tag="pj", bufs=2, name="projps")
                for half in range(2):
                    tk = ds((t8 % 4) * 512 + half * 256, 256)
                    o = ps[:, ts(half, 256)]
                    n = 0
                    for wsb, xsb in ((wh, xh_sb), (wh, xr_sb), (wr, xh_sb)):
                        for a in range(HC):
                            nc.tensor.matmul(
                                o,
                                wsb[:, a, :, :],
                                xsb[:, a, :, tk],
                                start=(n == 0),
                                stop=(n == 3 * HC - 1),
                                perf_mode=DR,
                            )
                            n += 1
                if which == "q":
                    nc.scalar.activation(
                        dst[:, ts(t8, 512)], ps, Ident, bias=bq_sb, scale=1.0 / 512.0
                    )
                else:
                    nc.scalar.activation(dst[:, ts(t8, 512)], ps, Copy, scale=1.0 / 512.0)

            def v_tile(t32):
                """V for one 128-token tile, direct [tok, col] layout."""
                wh, wr = w_sbs["whv"], w_sbs["wrv"]
                ps = psp.tile([P, P], f32, tag="ctx", bufs=2, name="vps")
                tk = ds((t32 % 16) * P, P)
                n = 0
                for wsb, xsb in ((wh, xh_sb), (wh, xr_sb), (wr, xh_sb)):
                    for a in range(HC):
                        nc.tensor.matmul(
                            ps,
                            xsb[:, a, :, tk],
                            wsb[:, a, :, :],
                            start=(n == 0),
                            stop=(n == 3 * HC - 1),
                            perf_mode=DR,
                        )
                        n += 1
                dst = vp_sb[:, t32, :, 0:D]
                nc.vector.tensor_scalar_mul(
                    dst, ps.rearrange("p (h d) -> p h d", h=2), 1.0 / 512.0
                )

            # PT ring: 2 heads x 32 slots x [128, 512] fp16 (2 units deep)
            RING = 32
            pt_all = ptpool.tile([P, 2, RING, 512], f16, tag="pt")

            ot_tiles = {}

            def pv_qc(unit, head, qc):
                """P@V for one 128-q chunk: ctx[q,65] += PT[k,qc].T @ V'."""
                b = unit // 4
                ctx = psp.tile([P, D + 1], f32, tag="ctx", bufs=2, name="ctx")
                for kt in range(16):
                    nc.tensor.matmul(
                        ctx,
                        pt_all[:, head, (unit * 16 + kt) % RING, ts(qc, P)],
                        vp_sb[:, b * 16 + kt, head, 0 : D + 1],
                        start=(kt == 0),
                        stop=(kt == 15),
                    )
                return ctx

            def pv_norm(ctx, unit, head, qc):
                """Normalize one ctx chunk into the (unit, head) staging tile;
                DMA the 512 q rows out after the last chunk."""
                qbase = (unit // 4) * S + (unit % 4) * 512
                key = (unit, head)
                if key not in ot_tiles:
                    ot_tiles[key] = otpool.tile([P, 4, D], f32, tag="ot", name="ot")
                ot = ot_tiles[key]
                rc = smallpool.tile([P, 1], f32, tag="rc")
                nc.vector.reciprocal(rc, ctx[:, D : D + 1])
                nc.scalar.activation(ot[:, qc, :], ctx[:, 0:D], Copy, scale=rc)
                if qc in (1, 3):
                    half = qc // 2
                    dst = out[
                        ds(qbase + half * 256, 256), ds(D * head, D)
                    ].rearrange("(qc p) d -> p qc d", p=P)
                    nc.sync.dma_start(dst, ot[:, ds(half * 2, 2), :])
                    if qc == 3:
                        del ot_tiles[key]

            def pv_full(unit, head, qc):
                ctx = pv_qc(unit, head, qc)
                pv_norm(ctx, unit, head, qc)

            # Deferred-work queue: (cost, fn, deadline). Deadline (u, kt)
            # means the item MUST be emitted before (u, kt)'s scores/exp.
            work_q = deque()

            def q_proj(t8, which, dl):
                work_q.append((1.28, lambda: proj_group(t8, which), dl))

            def q_v(tiles, dl):
                for t in tiles:
                    work_q.append((0.32, lambda tt=t: v_tile(tt), dl))

            def q_pv(unit, head, qcs, dl):
                for qc in qcs:
                    work_q.append(
                        (0.43, lambda h=head, q=qc: pv_full(unit, h, q), dl)
                    )

            def push_unit_work(unit):
                if unit == 0:
                    q_proj(1, "k", (0, 4))  # k1
                    q_proj(1, "q", (1, 0))  # q1
                    q_proj(2, "k", (0, 8))  # k2
                    q_v(range(0, 4), (1, 0))
                    q_proj(3, "k", (0, 12))  # k3
                    q_v(range(4, 8), (1, 0))
                elif unit == 1:
                    q_v(range(8, 16), (1, 12))
                    q_pv(0, 0, range(4), (2, 0))
                    q_proj(2, "q", (2, 0))  # q2
                    q_pv(0, 1, range(4), (2, 0))
                    q_proj(3, "q", (2, 0))  # q3
                elif unit == 2:
                    q_pv(1, 0, range(4), (3, 0))
                    q_proj(4, "k", (4, 0))  # k4
                    q_pv(1, 1, range(4), (3, 0))
                    q_proj(5, "k", (4, 4))  # k5
                    q_v(range(16, 20), (5, 0))
                elif unit == 3:
                    q_pv(2, 0, range(4), (4, 0))
                    q_proj(6, "k", (4, 8))  # k6
                    q_pv(2, 1, range(4), (4, 0))
                    q_proj(7, "k", (4, 12))  # k7
                    q_proj(4, "q", (4, 0))  # q4
                    q_v(range(20, 24), (5, 0))
                elif unit == 4:
                    q_pv(3, 0, range(4), (5, 0))
                    q_proj(5, "q", (5, 0))  # q5
                    q_pv(3, 1, range(4), (5, 0))
                    q_v(range(24, 32), (5, 0))
                elif unit == 5:
                    q_pv(4, 0, range(4), (6, 0))
                    q_proj(6, "q", (6, 0))  # q6
                    q_pv(4, 1, range(4), (6, 0))
                    q_proj(7, "q", (7, 0))  # q7
                elif unit == 6:
                    q_pv(5, 0, range(4), (7, 0))
                    q_pv(5, 1, range(4), (7, 0))
                elif unit == 7:
                    q_pv(6, 0, range(4), (8, 0))
                    q_pv(6, 1, range(4), (8, 0))

            # ---- batch 0 essentials: just enough for unit 0's scores ----
            proj_group(0, "q")
            proj_group(0, "k")

            for unit in range(8):
                b = unit // 4
                base = b * S
                qbase = base + (unit % 4) * 512
                if unit == 2:
                    # drain every batch-0 consumer of x first (emission
                    # order is semantic order), then reload X^T with batch 1
                    while work_q and work_q[0][2] <= (2, 0):
                        work_q.popleft()[1]()
                    for quarter in range(4):
                        nc.sync.dma_start(
                            xh_sb[:, :, :, ts(quarter, 512)],
                            xhp[:, :, :, ds(S + quarter * 512, 512)],
                        )
                        nc.sync.dma_start(
                            xr_sb[:, :, :, ts(quarter, 512)],
                            xrp[:, :, :, ds(S + quarter * 512, 512)],
                        )
                push_unit_work(unit)
                credit = 2.0
                for kt in range(16):
                    while work_q and work_q[0][2] <= (unit, kt):
                        _, fn, _ = work_q.popleft()
                        fn()
                    slot = (unit * 16 + kt) % RING
                    for head in range(2):
                        hb = D * head
                        st = psp.tile([P, 512], f32, tag="st", bufs=4, name="st")
                        nc.tensor.matmul(
                            st,
                            kt_sb[ds(hb, D), ds(base + kt * P, P)],
                            qt_sb[ds(hb, D), ds(qbase, 512)],
                            start=True,
                            stop=True,
                        )
                        dst = pt_all[:, head, slot, :]
                        if head in DVE_H.get(unit, ()):
                            nc.vector.tensor_scalar(
                                dst.bitcast(u16), st, A16, B16, MULT, ADD
                            )
                        else:
                            nc.scalar.activation(dst, st, Exp, scale=0.125)
                    # deferred work drained under the kt shadow, paced
                    credit = min(credit + 0.65, 6.0)
                    while work_q and work_q[0][0] <= credit:
                        cost, fn, _ = work_q.popleft()
                        credit -= cost
                        fn()
            while work_q:
                work_q.popleft()[1]()
            # unit 7's P@V has no later exp shadow: straight-line tail,
            # heads interleaved so norms/DMAs overlap remaining matmuls
            for qc in range(4):
                for head in range(2):
                    pv_full(7, head, qc)

    nc.compile()
    return nc


def get_nc():
    if "nc" not in _CACHED:
        _CACHED["nc"] = _build()
    return _CACHED["nc"]


def _wsplit(Wslice):
    """Two-term fp8 split of a weight column slice at common scale 512.

    Returns (wh16, wr) fp8 arrays with W ~= (wh16 + wr) / 512, where
    wh16 = fp8(W*32)*16 exactly (exponent shift) and wr = fp8(512*(W - fp8(W*32)/32)).
    """
    w32 = (Wslice * 32.0).astype(F8)
    wh16 = (w32.astype(np.float32) * 16.0).astype(F8)
    wr = ((Wslice - w32.astype(np.float32) / 32.0) * 512.0).astype(F8)
    return wh16, wr


def kernel(hidden_states, Wq, bq, Wk, bk, Wv, bv):
    from concourse.bass_utils import run_bass_kernel_spmd

    nc = get_nc()

    x2 = np.asarray(hidden_states, dtype=np.float32).reshape(T, HID)
    xt = np.ascontiguousarray(x2.T)
    xh = xt.astype(F8)
    xr = (xt - xh.astype(np.float32)).astype(F8)

    Wq = np.asarray(Wq, np.float32)
    Wk = np.asarray(Wk, np.float32)
    Wv = np.asarray(Wv, np.float32)
    bqf = np.asarray(bq, np.float32)
    bvf = np.asarray(bv, np.float32)

    in_maps = []
    for c in range(N_CORES):
        sl = slice(P * c, P * (c + 1))
        whq, wrq = _wsplit(Wq[:, sl])
        whk, wrk = _wsplit(Wk[:, sl])
        whv, wrv = _wsplit(Wv[:, sl])
        in_maps.append(
            {
                "xh": xh,
                "xr": xr,
                "whq": np.ascontiguousarray(whq),
                "wrq": np.ascontiguousarray(wrq),
                "whk": np.ascontiguousarray(whk),
                "wrk": np.ascontiguousarray(wrk),
                "whv": np.ascontiguousarray(whv),
                "wrv": np.ascontiguousarray(wrv),
                "bq": np.ascontiguousarray(bqf[sl][:, None]),
            }
        )

    res = run_bass_kernel_spmd(nc, in_maps, list(range(N_CORES)))

    full = np.empty((T, HID), dtype=np.float32)
    for c in range(N_CORES):
        full[:, P * c : P * (c + 1)] = res.results[c]["out"]
    full += bvf[None, :]
    return full.reshape(B, S, HID)
